# Initial kernel scaffold
#
"""Trainium2 Bass kernel for causal MHA (RoPE) — nn_MultiHeadAttention_84447646974458.

Sharding: 8 cores = 2 batches x 4 head-groups (tensor-parallel over heads).
Core c handles batch b=c//4, head group g=c%4 (heads 4g..4g+3).

Per-core dataflow (everything in "T layout" [feature, token] except V):
  - Projections: qT/kT [dh, tok] for the 4 local heads with RoPE applied
    via DVE partition-rotated copies (sign folded into the sin table);
    V [tok, oc]. The first q/k tiles run contraction-outer so the PE
    keeps pace with the input DMA stream (startup is DMA-bound).
  - Causal attention per 512-token query tile with scores transposed
    [tokk, tokq]; masked query columns of diagonal chunks are never
    computed; exp on ACT (no max subtraction — scores are O(5));
    probs accumulate on DVE so the denominator is ONE ones-matmul per
    head; normalization (reciprocal_approx_fast + bf16 broadcast matmul
    + DVE mul) lagged one head so the PE never waits. Each tile's
    outputs are AllGathered (4-core groups).
  - Schedule: projections run one tile AHEAD of attention
    (qkv0,qkv1,attn0,qkv2,attn1,qkv3,attn2,attn3) — attention(tq) only
    attends keys <= tile tq, and spreading the AllGathers across the
    projection span keeps the serial CC stream (15-37us per op) off the
    critical path. o_proj (column-sharded: all tokens x own 512 cols)
    is emitted last so the kernel tail is pure matmul work, reading the
    gathered heads from recycled hid SBUF slots.
Host reassembles out[b, :, 512g:512(g+1)] from core (b,g).
"""
import math
import numpy as np
import ml_dtypes

import concourse.bass as bass
import concourse.tile as tile
from concourse import bacc, mybir
from concourse.bass_utils import run_bass_kernel_spmd

F32 = mybir.dt.float32
BF16 = mybir.dt.bfloat16

B, S, H = 2, 2048, 2048
NH, DH = 16, 128
HPG = 4            # heads per group (per core)
OCG = HPG * DH     # 512 channels per group
NC = 8
SCALE = 1.0 / math.sqrt(DH)
THETA = 10000.0

TQ = 512           # query-token tile (free dim of attention matmuls)
KC = H // 128      # 16 contraction chunks of 128


def _rope_tables(s):
    invf = 1.0 / (THETA ** (np.arange(0, DH, 2, dtype=np.float32) / DH))
    t = np.arange(s, dtype=np.float32)
    fr = np.concatenate([np.outer(t, invf)] * 2, axis=1)  # [s, DH]
    cosT = np.cos(fr).T.copy()                            # [DH, s]
    ssinT = np.sin(fr).T.copy()
    ssinT[:DH // 2] *= -1.0       # sign of rotate-half folded into the table
    return cosT, ssinT


def build_nc(s=S, num_devices=NC, groups=None):
    n_tq = s // TQ
    nc = bacc.Bacc("TRN2", target_bir_lowering=False, debug=False,
                   num_devices=num_devices)

    hidT = nc.dram_tensor("hidT", [H, s], BF16, kind="ExternalInput")
    wqT = nc.dram_tensor("wqT", [H, OCG], BF16, kind="ExternalInput")
    wkT = nc.dram_tensor("wkT", [H, OCG], BF16, kind="ExternalInput")
    wvT = nc.dram_tensor("wvT", [H, OCG], BF16, kind="ExternalInput")
    woT = nc.dram_tensor("woT", [H, OCG], BF16, kind="ExternalInput")
    out = nc.dram_tensor("out", [s, OCG], BF16, kind="ExternalOutput")

    # ---- host-computed constants (embedded in NEFF) ----
    cosT, ssinT = _rope_tables(s)
    # causal triangle mask tri[r, c] = (r <= c): a diagonal 128-key chunk
    # against a 512-query block is [all-zero cols | this triangle | all-one
    # cols], so only a [128,128] block ever needs a mask multiply.
    maskM = (np.arange(128)[:, None] <= np.arange(128)[None, :]).astype(np.float32)

    cosT_d = nc.inline_tensor(cosT.astype(ml_dtypes.bfloat16), name="cosT")
    ssinT_d = nc.inline_tensor(ssinT.astype(ml_dtypes.bfloat16), name="ssinT")
    maskM_d = nc.inline_tensor(maskM.astype(ml_dtypes.bfloat16), name="maskM")
    ones_d = nc.inline_tensor(np.ones((128, 1), ml_dtypes.bfloat16), name="onesc")
    ones1_d = nc.inline_tensor(np.ones((1, 128), ml_dtypes.bfloat16), name="ones1")

    if groups is None:
        groups = [[0, 1, 2, 3], [4, 5, 6, 7]] if num_devices == 8 else [list(range(num_devices))]
    n_group = len(groups[0])

    with tile.TileContext(nc) as tc:
        with (
            tc.tile_pool(name="consts", bufs=1) as pc,
            tc.tile_pool(name="weights", bufs=1) as pw,
            tc.tile_pool(name="hid", bufs=1) as ph,
            tc.tile_pool(name="acts", bufs=1) as pa,
            tc.tile_pool(name="work", bufs=1) as pk,
            tc.tile_pool(name="probs", bufs=1) as pp,
            tc.tile_pool(name="psum", bufs=1, space="PSUM") as ps,
            tc.tile_pool(name="dram", bufs=1, space="DRAM") as pd,
        ):
            # ---- input loads, ordered to match first-use (startup is
            # DMA-bound): rope tables, then wq/hid chunk pairs (q tile 0
            # consumes them in this order), wk, wv, attention consts.
            def load_w(src_t, name, emit=True):
                t = pw.tile([128, KC * OCG], BF16, tag="w", bufs=3, name=name)
                if emit:
                    for hh in range(KC):
                        nc.sync.dma_start(t[:, hh * OCG:(hh + 1) * OCG],
                                          src_t[hh * 128:(hh + 1) * 128, :])
                return t

            # hid streams in two token-halves: projections for tiles 0/1 only
            # read tokens 0:1024, so the first half unblocks them 2x sooner.
            wq_sb = load_w(wqT, "wq", emit=False)
            hid_sb = [ph.tile([128, s], BF16, name=f"hid{hh}", tag="hid",
                              bufs=KC) for hh in range(KC)]
            h2 = s // 2
            for hh in range(KC):
                nc.sync.dma_start(wq_sb[:, hh * OCG:(hh + 1) * OCG],
                                  wqT[hh * 128:(hh + 1) * 128, :])
                nc.sync.dma_start(hid_sb[hh][:, 0:h2],
                                  hidT[hh * 128:(hh + 1) * 128, 0:h2])
            # rope tables ride the (idle) Activation DMA queue so they land
            # well before the first rope without delaying the wq/hid stream
            cos_sb = pc.tile([DH, s], BF16)
            nc.scalar.dma_start(cos_sb[:], cosT_d[:])
            ssin_sb = pc.tile([DH, s], BF16)
            nc.scalar.dma_start(ssin_sb[:], ssinT_d[:])
            wk_sb = load_w(wkT, "wk")
            wv_sb = load_w(wvT, "wv")
            for hh in range(KC):
                nc.sync.dma_start(hid_sb[hh][:, h2:s],
                                  hidT[hh * 128:(hh + 1) * 128, h2:s])
            mask_sb = pc.tile([128, 128], BF16)
            nc.sync.dma_start(mask_sb[:], maskM_d[:])
            ones_sb = pc.tile([128, 1], BF16)
            nc.sync.dma_start(ones_sb[:], ones_d[:])
            ones1_sb = pc.tile([1, 128], BF16)
            nc.sync.dma_start(ones1_sb[:], ones1_d[:])

            qT_sb = pa.tile([128, HPG * s], BF16, name="qT")
            kT_sb = pa.tile([128, HPG * s], BF16, name="kT")
            v_sb = pa.tile([128, (s // 128) * OCG], BF16, name="v")

            def rope(pm, dst, tq):
                """dst[:, :TQ] = pm*cos + rot_half(pm)*ssin (DVE only)."""
                c0, c1 = tq * TQ, (tq + 1) * TQ
                a = pk.tile([128, TQ], F32, tag="ra", bufs=1)
                nc.vector.tensor_mul(a[:], pm[:], cos_sb[:, c0:c1])
                rot = pk.tile([128, TQ], F32, tag="rr", bufs=1)
                nc.vector.tensor_copy(rot[0:64, :], pm[64:128, :])
                nc.vector.tensor_copy(rot[64:128, :], pm[0:64, :])
                b = pk.tile([128, TQ], F32, tag="rb", bufs=1)
                nc.vector.tensor_mul(b[:], rot[:], ssin_sb[:, c0:c1])
                nc.vector.tensor_add(dst, a[:], b[:])

            def qk_tile(w_sb, dst_sb, j, tq):
                """One [128ch x 512tok] projection chain + RoPE (j-outer)."""
                pm = ps.tile([128, TQ], F32, tag="mm", bufs=2)
                for hh in range(KC):
                    nc.tensor.matmul(
                        pm[:],
                        w_sb[:, hh * OCG + j * 128: hh * OCG + (j + 1) * 128],
                        hid_sb[hh][:, tq * TQ:(tq + 1) * TQ],
                        start=(hh == 0), stop=(hh == KC - 1))
                rope(pm, dst_sb[:, j * s + tq * TQ: j * s + (tq + 1) * TQ], tq)

            def qk_tile0(w_sb, dst_sb):
                """tq=0 projection with contraction outer (DMA pacing)."""
                pms = [ps.tile([128, TQ], F32, tag=t, bufs=2, name=f"p0{i}")
                       for i, t in enumerate(("mm", "mm", "st", "st"))]
                for hh in range(KC):
                    for j in range(HPG):
                        nc.tensor.matmul(
                            pms[j][:],
                            w_sb[:, hh * OCG + j * 128: hh * OCG + (j + 1) * 128],
                            hid_sb[hh][:, 0:TQ],
                            start=(hh == 0), stop=(hh == KC - 1))
                        if hh == KC - 1:
                            rope(pms[j], dst_sb[:, j * s: j * s + TQ], 0)

            def v_tile(tcch):
                pm = ps.tile([128, OCG], F32, tag="o", bufs=2)
                for hh in range(KC):
                    nc.tensor.matmul(
                        pm[:],
                        hid_sb[hh][:, tcch * 128:(tcch + 1) * 128],
                        wv_sb[:, hh * OCG:(hh + 1) * OCG],
                        start=(hh == 0), stop=(hh == KC - 1))
                nc.scalar.activation(v_sb[:, tcch * OCG:(tcch + 1) * OCG], pm[:],
                                     mybir.ActivationFunctionType.Copy)

            def qk(tq):
                if tq == 0:
                    qk_tile0(wq_sb, qT_sb)
                    qk_tile0(wk_sb, kT_sb)
                else:
                    for j in range(HPG):
                        qk_tile(wq_sb, qT_sb, j, tq)
                    for j in range(HPG):
                        qk_tile(wk_sb, kT_sb, j, tq)

            def vproj(tq):
                for sub in range(TQ // 128):
                    v_tile(tq * (TQ // 128) + sub)

            # ============ attention + AllGather per tile ============
            ag_ins = [pd.tile([OCG, TQ], BF16, name=f"agi{t}", tag="agi",
                              bufs=n_tq) for t in range(n_tq)]
            ag_outs = [pd.tile([n_group * OCG, TQ], BF16, name=f"ago{t}",
                               tag="ago", bufs=n_tq) for t in range(n_tq)]

            def norm_head(po, psm, ots, hd, tq):
                """ot = po / broadcast(sum) — lagged off the PE critical path."""
                po_sb = pk.tile([128, TQ], F32, tag="posb", bufs=2)
                nc.scalar.activation(po_sb[:], po[:],
                                     mybir.ActivationFunctionType.Copy)
                recip = pk.tile([1, TQ], F32, tag="rc", bufs=2)
                nc.vector.reciprocal_approx_fast(recip[:], psm[:])
                recb = pk.tile([1, TQ], BF16, tag="rcb", bufs=2)
                nc.vector.tensor_copy(recb[:], recip[:])
                bc = ps.tile([128, TQ], F32, tag="bc", bufs=1)
                nc.tensor.matmul(bc[:], ones1_sb[:], recb[:],
                                 start=True, stop=True)
                ot = pk.tile([128, TQ], BF16, tag="ot", bufs=2 * HPG,
                             name=f"ot{tq}_{hd}")
                nc.vector.tensor_mul(ot[:], po_sb[:], bc[:])
                nc.sync.dma_start(ag_ins[tq][hd * 128:(hd + 1) * 128, :], ot[:])
                ots[hd] = ot

            def attention(tq, fillers=()):
                nkk = HPG * (tq + 1)
                ots = [None] * HPG
                lag = []
                fill_iter = iter(fillers)

                def scores(hd, kk):
                    # Diagonal chunk j: query cols < 128j are fully masked —
                    # never compute them. Scores/exp/AV/acc all operate on
                    # cols [c0:512]; the triangular 128-col block gets a DVE
                    # mask multiply; stale pr cols below c0 are never read.
                    j = kk - HPG * tq
                    c0 = 128 * j if j > 0 else 0
                    st = ps.tile([128, TQ], F32, tag="st", bufs=2)
                    nc.tensor.matmul(
                        st[:, c0:],
                        kT_sb[:, hd * s + kk * 128: hd * s + (kk + 1) * 128],
                        qT_sb[:, hd * s + tq * TQ + c0: hd * s + (tq + 1) * TQ],
                        start=True, stop=True)
                    pr = pp.tile([128, TQ], BF16, tag="pr", bufs=5)
                    nc.scalar.activation(pr[:, c0:], st[:, c0:],
                                         mybir.ActivationFunctionType.Exp,
                                         scale=SCALE)
                    if j >= 0:
                        nc.vector.tensor_mul(
                            pr[:, c0:c0 + 128],
                            pr[:, c0:c0 + 128], mask_sb[:])
                    return pr, c0

                for hd in range(HPG):
                    po = ps.tile([128, TQ], F32, tag="o", bufs=2)
                    # Probs accumulate on DVE (bf16); the denominator is ONE
                    # ones-matmul on the accumulated sum instead of one per
                    # key chunk — saves ~150 PE matmuls per kernel.
                    acc = pk.tile([128, TQ], BF16, tag="acc", bufs=2)
                    pr_next = scores(hd, 0)
                    for kk in range(nkk):
                        pr, c0 = pr_next
                        if kk + 1 < nkk:
                            pr_next = scores(hd, kk + 1)
                        nc.tensor.matmul(
                            po[:, c0:],
                            v_sb[:, kk * OCG + hd * 128: kk * OCG + (hd + 1) * 128],
                            pr[:, c0:], start=(kk == 0),
                            stop=(kk == nkk - 1), skip_group_check=True)
                        if kk == 0:
                            nc.vector.tensor_copy(acc[:], pr[:])
                        else:
                            nc.vector.tensor_add(acc[:, c0:], acc[:, c0:],
                                                 pr[:, c0:])
                    psm = ps.tile([1, TQ], F32, tag="sum", bufs=1)
                    nc.tensor.matmul(psm[:], ones_sb[:], acc[:],
                                     start=True, stop=True)
                    lag.append((po, psm, hd))
                    if hd > 0:
                        p_, s_, h_ = lag.pop(0)
                        norm_head(p_, s_, ots, h_, tq)
                    # head boundary: slip in projection chains for a later
                    # tile — pure PE work that lets the exp pipeline drain
                    f = next(fill_iter, None)
                    if f:
                        f()
                p_, s_, h_ = lag.pop(0)
                norm_head(p_, s_, ots, h_, tq)
                nc.gpsimd.collective_compute(
                    "AllGather", mybir.AluOpType.bypass,
                    replica_groups=groups,
                    ins=[ag_ins[tq][:].opt()], outs=[ag_outs[tq][:].opt()])

            def readback(tq):
                """ag_outs -> recycled hid slots. Emitted only once the hid
                slots' last projection reader exists (WAR), so the triggers
                never block the sync queue mid-pipeline."""
                at_sb = []
                for sl in range(HPG):
                    t = ph.tile([128, s], BF16, tag="hid", bufs=KC,
                                name=f"at{tq}_{sl}")
                    nc.sync.dma_start(
                        t[:].rearrange("p (a c) -> p a c", a=4),
                        ag_outs[tq][sl * 512:(sl + 1) * 512, :]
                        .rearrange("(a p) c -> p a c", a=4))
                    at_sb.append(t)
                return at_sb

            # Interleave projections one tile AHEAD of attention: spreads the
            # AllGathers across the whole projection span (the CC stream
            # serializes ops at 15-37us each, so bunching them at the end
            # stalls the last tile's gather), and attention(tq) only needs
            # keys/values up to tile tq anyway.
            qk(0)
            vproj(0)
            qk(1)
            vproj(1)
            attention(0)
            qk(2)
            vproj(2)
            qk3_fill = ([(lambda j=j: qk_tile(wq_sb, qT_sb, j, 3))
                         for j in range(HPG)] +
                        [(lambda j=j: qk_tile(wk_sb, kT_sb, j, 3))
                         for j in range(HPG)])
            attention(1, fillers=qk3_fill[:4])
            attention(2, fillers=qk3_fill[4:])
            vproj(3)
            # wo reuses wq's SBUF slot; triggers ride the Activation queue so
            # their wait (wq's last reader = qk(3)) can't block sync DMAs.
            wo_sb = load_w(woT, "wo", emit=False)
            for hh in range(KC):
                nc.scalar.dma_start(wo_sb[:, hh * OCG:(hh + 1) * OCG],
                                    woT[hh * 128:(hh + 1) * 128, :])
            at_tiles = [readback(0), readback(1)]
            attention(3)
            at_tiles += [readback(2), readback(3)]

            # ============ phase 3: o_proj (column-sharded, PE-only tail) ====
            # All inputs (at_tiles, readbacks issued during phase 2) are in
            # SBUF by now; this is pure PE work so the kernel tail is short.
            for tq in range(n_tq):
                at_sb = at_tiles[tq]
                for sub in range(TQ // 128):
                    pm = ps.tile([128, OCG], F32, tag="mm", bufs=2)
                    for i in range(n_group * HPG):
                        nc.tensor.matmul(
                            pm[:],
                            at_sb[i // 4][:, (i % 4) * TQ + sub * 128:
                                          (i % 4) * TQ + (sub + 1) * 128],
                            wo_sb[:, i * OCG:(i + 1) * OCG],
                            start=(i == 0), stop=(i == n_group * HPG - 1))
                    ob = pk.tile([128, OCG], BF16, tag="ob", bufs=6)
                    nc.vector.tensor_copy(ob[:], pm[:])
                    r0 = tq * TQ + sub * 128
                    # Activation HWDGE queue: the sync queue's tail is the
                    # at(3) readback triggers still waiting on AG(3).
                    nc.scalar.dma_start(out[r0:r0 + 128, :], ob[:])

    nc.compile()
    return nc


_NC_CACHE = {}


def _get_nc():
    if "nc" not in _NC_CACHE:
        _NC_CACHE["nc"] = build_nc()
    return _NC_CACHE["nc"]


def _build_in_maps(hidden_states, w_qkv, w_o):
    bf = ml_dtypes.bfloat16
    hidT = [np.ascontiguousarray(hidden_states[b].T).astype(bf) for b in range(B)]
    wq_all = w_qkv[:H].T.astype(bf)
    wk_all = w_qkv[H:2 * H].T.astype(bf)
    wv_all = w_qkv[2 * H:].T.astype(bf)
    wo_all = w_o.T.astype(bf)
    in_maps = []
    for c in range(NC):
        b, g = c // 4, c % 4
        sl = slice(g * OCG, (g + 1) * OCG)
        in_maps.append({
            "hidT": hidT[b],
            "wqT": np.ascontiguousarray(wq_all[:, sl]),
            "wkT": np.ascontiguousarray(wk_all[:, sl]),
            "wvT": np.ascontiguousarray(wv_all[:, sl]),
            "woT": np.ascontiguousarray(wo_all[:, sl]),
        })
    return in_maps


def kernel(hidden_states, w_qkv, w_o):
    hidden_states = np.asarray(hidden_states, dtype=np.float32)
    w_qkv = np.asarray(w_qkv, dtype=np.float32)
    w_o = np.asarray(w_o, dtype=np.float32)

    nc = _get_nc()
    in_maps = _build_in_maps(hidden_states, w_qkv, w_o)
    res = run_bass_kernel_spmd(nc, in_maps, core_ids=list(range(NC)))

    out = np.empty((B, S, H), np.float32)
    for c in range(NC):
        b, g = c // 4, c % 4
        out[b, :, g * OCG:(g + 1) * OCG] = \
            np.asarray(res.results[c]["out"], dtype=np.float32)
    return out



# revision 1
# speedup vs baseline: 1.2962x; 1.2962x over previous
"""Trainium2 Bass kernel for causal MHA (RoPE) — nn_MultiHeadAttention_84447646974458.

Sharding: 8 cores = 2 batches x 4 head-groups (tensor-parallel over heads).
Core c handles batch b=c//4, head group g=c%4 (heads 4g..4g+3).

Per-core dataflow (everything in "T layout" [feature, token] except V):
  - Projections: qT/kT [dh, tok] for the 4 local heads with RoPE applied
    via DVE partition-rotated copies (sign folded into the sin table);
    V [tok, oc]. The first q/k tiles run contraction-outer so the PE
    keeps pace with the input DMA stream (startup is DMA-bound).
  - Causal attention per 512-token query tile with scores transposed
    [tokk, tokq]; masked query columns of diagonal chunks are never
    computed; exp on ACT (no max subtraction — scores are O(5));
    probs accumulate on DVE so the denominator is ONE ones-matmul per
    head; normalization (reciprocal_approx_fast + bf16 broadcast matmul
    + DVE mul) lagged one head so the PE never waits. Each tile's
    outputs are AllGathered (4-core groups).
  - Schedule: projections run one tile AHEAD of attention
    (qkv0,qkv1,attn0,qkv2,attn1,qkv3,attn2,attn3) — attention(tq) only
    attends keys <= tile tq, and spreading the AllGathers across the
    projection span keeps the serial CC stream (15-37us per op) off the
    critical path. o_proj (column-sharded: all tokens x own 512 cols)
    is emitted last so the kernel tail is pure matmul work, reading the
    gathered heads from recycled hid SBUF slots.
Host reassembles out[b, :, 512g:512(g+1)] from core (b,g).
"""
import math
import numpy as np
import ml_dtypes

import concourse.bass as bass
import concourse.tile as tile
from concourse import bacc, mybir
from concourse.bass_utils import run_bass_kernel_spmd

F32 = mybir.dt.float32
BF16 = mybir.dt.bfloat16

B, S, H = 2, 2048, 2048
NH, DH = 16, 128
HPG = 4            # heads per group (per core)
OCG = HPG * DH     # 512 channels per group
NC = 8
SCALE = 1.0 / math.sqrt(DH)
THETA = 10000.0

TQ = 512           # query-token tile (free dim of attention matmuls)
KC = H // 128      # 16 contraction chunks of 128


def _rope_tables(s):
    invf = 1.0 / (THETA ** (np.arange(0, DH, 2, dtype=np.float32) / DH))
    t = np.arange(s, dtype=np.float32)
    fr = np.concatenate([np.outer(t, invf)] * 2, axis=1)  # [s, DH]
    cosT = np.cos(fr).T.copy()                            # [DH, s]
    ssinT = np.sin(fr).T.copy()
    ssinT[:DH // 2] *= -1.0       # sign of rotate-half folded into the table
    return cosT, ssinT


def build_nc(s=S, num_devices=NC, groups=None):
    n_tq = s // TQ
    nc = bacc.Bacc("TRN2", target_bir_lowering=False, debug=False,
                   num_devices=num_devices)

    hidT = nc.dram_tensor("hidT", [H, s], BF16, kind="ExternalInput")
    wqT = nc.dram_tensor("wqT", [H, OCG], BF16, kind="ExternalInput")
    wkT = nc.dram_tensor("wkT", [H, OCG], BF16, kind="ExternalInput")
    wvT = nc.dram_tensor("wvT", [H, OCG], BF16, kind="ExternalInput")
    woT = nc.dram_tensor("woT", [H, OCG], BF16, kind="ExternalInput")
    out = nc.dram_tensor("out", [s, OCG], BF16, kind="ExternalOutput")

    # ---- host-computed constants (embedded in NEFF) ----
    cosT, ssinT = _rope_tables(s)
    # causal triangle mask tri[r, c] = (r <= c): a diagonal 128-key chunk
    # against a 512-query block is [all-zero cols | this triangle | all-one
    # cols], so only a [128,128] block ever needs a mask multiply.
    maskM = (np.arange(128)[:, None] <= np.arange(128)[None, :]).astype(np.float32)

    cosT_d = nc.inline_tensor(cosT.astype(ml_dtypes.bfloat16), name="cosT")
    ssinT_d = nc.inline_tensor(ssinT.astype(ml_dtypes.bfloat16), name="ssinT")
    maskM_d = nc.inline_tensor(maskM.astype(ml_dtypes.bfloat16), name="maskM")
    ones_d = nc.inline_tensor(np.ones((128, 1), ml_dtypes.bfloat16), name="onesc")
    ones1_d = nc.inline_tensor(np.ones((1, 128), ml_dtypes.bfloat16), name="ones1")

    if groups is None:
        groups = [[0, 1, 2, 3], [4, 5, 6, 7]] if num_devices == 8 else [list(range(num_devices))]
    n_group = len(groups[0])

    with tile.TileContext(nc) as tc:
        with (
            tc.tile_pool(name="consts", bufs=1) as pc,
            tc.tile_pool(name="weights", bufs=1) as pw,
            tc.tile_pool(name="hid", bufs=1) as ph,
            tc.tile_pool(name="acts", bufs=1) as pa,
            tc.tile_pool(name="work", bufs=1) as pk,
            tc.tile_pool(name="probs", bufs=1) as pp,
            tc.tile_pool(name="psum", bufs=1, space="PSUM") as ps,
            tc.tile_pool(name="dram", bufs=1, space="DRAM") as pd,
        ):
            # ---- input loads, ordered to match first-use (startup is
            # DMA-bound): rope tables, then wq/hid chunk pairs (q tile 0
            # consumes them in this order), wk, wv, attention consts.
            def load_w(src_t, name, emit=True):
                t = pw.tile([128, KC * OCG], BF16, tag="w", bufs=3, name=name)
                if emit:
                    for hh in range(KC):
                        nc.sync.dma_start(t[:, hh * OCG:(hh + 1) * OCG],
                                          src_t[hh * 128:(hh + 1) * 128, :])
                return t

            # hid streams in two token-halves: projections for tiles 0/1 only
            # read tokens 0:1024, so the first half unblocks them 2x sooner.
            wq_sb = load_w(wqT, "wq", emit=False)
            hid_sb = [ph.tile([128, s], BF16, name=f"hid{hh}", tag="hid",
                              bufs=KC) for hh in range(KC)]
            h2 = s // 2
            for hh in range(KC):
                nc.sync.dma_start(wq_sb[:, hh * OCG:(hh + 1) * OCG],
                                  wqT[hh * 128:(hh + 1) * 128, :])
                nc.sync.dma_start(hid_sb[hh][:, 0:h2],
                                  hidT[hh * 128:(hh + 1) * 128, 0:h2])
            # rope tables ride the (idle) Activation DMA queue so they land
            # well before the first rope without delaying the wq/hid stream
            cos_sb = pc.tile([DH, s], BF16)
            nc.scalar.dma_start(cos_sb[:], cosT_d[:])
            ssin_sb = pc.tile([DH, s], BF16)
            nc.scalar.dma_start(ssin_sb[:], ssinT_d[:])
            wk_sb = load_w(wkT, "wk")
            wv_sb = load_w(wvT, "wv")
            for hh in range(KC):
                nc.sync.dma_start(hid_sb[hh][:, h2:s],
                                  hidT[hh * 128:(hh + 1) * 128, h2:s])
            mask_sb = pc.tile([128, 128], BF16)
            nc.sync.dma_start(mask_sb[:], maskM_d[:])
            ones_sb = pc.tile([128, 1], BF16)
            nc.sync.dma_start(ones_sb[:], ones_d[:])
            ones1_sb = pc.tile([1, 128], BF16)
            nc.sync.dma_start(ones1_sb[:], ones1_d[:])

            qT_sb = pa.tile([128, HPG * s], BF16, name="qT")
            kT_sb = pa.tile([128, HPG * s], BF16, name="kT")
            v_sb = pa.tile([128, (s // 128) * OCG], BF16, name="v")

            def rope(pm, dst, tq):
                """dst[:, :TQ] = pm*cos + rot_half(pm)*ssin (DVE only)."""
                c0, c1 = tq * TQ, (tq + 1) * TQ
                a = pk.tile([128, TQ], F32, tag="ra", bufs=1)
                nc.vector.tensor_mul(a[:], pm[:], cos_sb[:, c0:c1])
                rot = pk.tile([128, TQ], F32, tag="rr", bufs=1)
                nc.vector.tensor_copy(rot[0:64, :], pm[64:128, :])
                nc.vector.tensor_copy(rot[64:128, :], pm[0:64, :])
                b = pk.tile([128, TQ], F32, tag="rb", bufs=1)
                nc.vector.tensor_mul(b[:], rot[:], ssin_sb[:, c0:c1])
                nc.vector.tensor_add(dst, a[:], b[:])

            def qk_tile(w_sb, dst_sb, j, tq):
                """One [128ch x 512tok] projection chain + RoPE (j-outer)."""
                pm = ps.tile([128, TQ], F32, tag="mm", bufs=2)
                for hh in range(KC):
                    nc.tensor.matmul(
                        pm[:],
                        w_sb[:, hh * OCG + j * 128: hh * OCG + (j + 1) * 128],
                        hid_sb[hh][:, tq * TQ:(tq + 1) * TQ],
                        start=(hh == 0), stop=(hh == KC - 1))
                rope(pm, dst_sb[:, j * s + tq * TQ: j * s + (tq + 1) * TQ], tq)

            def qk_tile0(w_sb, dst_sb):
                """tq=0 projection with contraction outer (DMA pacing)."""
                pms = [ps.tile([128, TQ], F32, tag=t, bufs=2, name=f"p0{i}")
                       for i, t in enumerate(("mm", "mm", "st", "st"))]
                for hh in range(KC):
                    for j in range(HPG):
                        nc.tensor.matmul(
                            pms[j][:],
                            w_sb[:, hh * OCG + j * 128: hh * OCG + (j + 1) * 128],
                            hid_sb[hh][:, 0:TQ],
                            start=(hh == 0), stop=(hh == KC - 1))
                        if hh == KC - 1:
                            rope(pms[j], dst_sb[:, j * s: j * s + TQ], 0)

            def v_tile(tcch):
                pm = ps.tile([128, OCG], F32, tag="o", bufs=2)
                for hh in range(KC):
                    nc.tensor.matmul(
                        pm[:],
                        hid_sb[hh][:, tcch * 128:(tcch + 1) * 128],
                        wv_sb[:, hh * OCG:(hh + 1) * OCG],
                        start=(hh == 0), stop=(hh == KC - 1))
                nc.scalar.activation(v_sb[:, tcch * OCG:(tcch + 1) * OCG], pm[:],
                                     mybir.ActivationFunctionType.Copy)

            def qk(tq):
                if tq == 0:
                    qk_tile0(wq_sb, qT_sb)
                    qk_tile0(wk_sb, kT_sb)
                else:
                    for j in range(HPG):
                        qk_tile(wq_sb, qT_sb, j, tq)
                    for j in range(HPG):
                        qk_tile(wk_sb, kT_sb, j, tq)

            def vproj(tq):
                for sub in range(TQ // 128):
                    v_tile(tq * (TQ // 128) + sub)

            # ============ attention + AllGather per tile ============
            ag_ins = [pd.tile([OCG, TQ], BF16, name=f"agi{t}", tag="agi",
                              bufs=n_tq) for t in range(n_tq)]
            ag_outs = [pd.tile([n_group * OCG, TQ], BF16, name=f"ago{t}",
                               tag="ago", bufs=n_tq) for t in range(n_tq)]

            def norm_head(po, psm, ots, hd, tq):
                """ot = po / broadcast(sum) — lagged off the PE critical path."""
                po_sb = pk.tile([128, TQ], F32, tag="posb", bufs=2)
                nc.scalar.activation(po_sb[:], po[:],
                                     mybir.ActivationFunctionType.Copy)
                recip = pk.tile([1, TQ], F32, tag="rc", bufs=2)
                nc.vector.reciprocal_approx_fast(recip[:], psm[:])
                recb = pk.tile([1, TQ], BF16, tag="rcb", bufs=2)
                nc.vector.tensor_copy(recb[:], recip[:])
                bc = ps.tile([128, TQ], F32, tag="bc", bufs=1)
                nc.tensor.matmul(bc[:], ones1_sb[:], recb[:],
                                 start=True, stop=True)
                ot = pk.tile([128, TQ], BF16, tag="ot", bufs=2 * HPG,
                             name=f"ot{tq}_{hd}")
                nc.vector.tensor_mul(ot[:], po_sb[:], bc[:])
                nc.sync.dma_start(ag_ins[tq][hd * 128:(hd + 1) * 128, :], ot[:])
                ots[hd] = ot

            def attention(tq, fillers=()):
                nkk = HPG * (tq + 1)
                ots = [None] * HPG
                lag = []
                fill_iter = iter(fillers)

                def scores(hd, kk):
                    # Diagonal chunk j: query cols < 128j are fully masked —
                    # never compute them. Scores/exp/AV/acc all operate on
                    # cols [c0:512]; the triangular 128-col block gets a DVE
                    # mask multiply; stale pr cols below c0 are never read.
                    j = kk - HPG * tq
                    c0 = 128 * j if j > 0 else 0
                    st = ps.tile([128, TQ], F32, tag="st", bufs=2)
                    nc.tensor.matmul(
                        st[:, c0:],
                        kT_sb[:, hd * s + kk * 128: hd * s + (kk + 1) * 128],
                        qT_sb[:, hd * s + tq * TQ + c0: hd * s + (tq + 1) * TQ],
                        start=True, stop=True)
                    pr = pp.tile([128, TQ], BF16, tag="pr", bufs=5)
                    nc.scalar.activation(pr[:, c0:], st[:, c0:],
                                         mybir.ActivationFunctionType.Exp,
                                         scale=SCALE)
                    if j >= 0:
                        nc.vector.tensor_mul(
                            pr[:, c0:c0 + 128],
                            pr[:, c0:c0 + 128], mask_sb[:])
                    return pr, c0

                for hd in range(HPG):
                    po = ps.tile([128, TQ], F32, tag="o", bufs=2)
                    # Probs accumulate on DVE (bf16); the denominator is ONE
                    # ones-matmul on the accumulated sum instead of one per
                    # key chunk — saves ~150 PE matmuls per kernel.
                    acc = pk.tile([128, TQ], BF16, tag="acc", bufs=2)
                    pr_next = scores(hd, 0)
                    for kk in range(nkk):
                        pr, c0 = pr_next
                        if kk + 1 < nkk:
                            pr_next = scores(hd, kk + 1)
                        nc.tensor.matmul(
                            po[:, c0:],
                            v_sb[:, kk * OCG + hd * 128: kk * OCG + (hd + 1) * 128],
                            pr[:, c0:], start=(kk == 0),
                            stop=(kk == nkk - 1), skip_group_check=True)
                        if kk == 0:
                            nc.vector.tensor_copy(acc[:], pr[:])
                        else:
                            nc.vector.tensor_add(acc[:, c0:], acc[:, c0:],
                                                 pr[:, c0:])
                    psm = ps.tile([1, TQ], F32, tag="sum", bufs=1)
                    nc.tensor.matmul(psm[:], ones_sb[:], acc[:],
                                     start=True, stop=True)
                    lag.append((po, psm, hd))
                    if hd > 0:
                        p_, s_, h_ = lag.pop(0)
                        norm_head(p_, s_, ots, h_, tq)
                    # head boundary: slip in projection chains for a later
                    # tile — pure PE work that lets the exp pipeline drain
                    f = next(fill_iter, None)
                    if f:
                        f()
                p_, s_, h_ = lag.pop(0)
                norm_head(p_, s_, ots, h_, tq)
                nc.gpsimd.collective_compute(
                    "AllGather", mybir.AluOpType.bypass,
                    replica_groups=groups,
                    ins=[ag_ins[tq][:].opt()], outs=[ag_outs[tq][:].opt()])

            def readback(tq):
                """ag_outs -> recycled hid slots. Emitted only once the hid
                slots' last projection reader exists (WAR), so the triggers
                never block the sync queue mid-pipeline."""
                at_sb = []
                for sl in range(HPG):
                    t = ph.tile([128, s], BF16, tag="hid", bufs=KC,
                                name=f"at{tq}_{sl}")
                    nc.sync.dma_start(
                        t[:].rearrange("p (a c) -> p a c", a=4),
                        ag_outs[tq][sl * 512:(sl + 1) * 512, :]
                        .rearrange("(a p) c -> p a c", a=4))
                    at_sb.append(t)
                return at_sb

            # Interleave projections one tile AHEAD of attention: spreads the
            # AllGathers across the whole projection span (the CC stream
            # serializes ops at 15-37us each, so bunching them at the end
            # stalls the last tile's gather), and attention(tq) only needs
            # keys/values up to tile tq anyway.
            qk(0)
            vproj(0)
            qk(1)
            vproj(1)
            attention(0)
            qk(2)
            vproj(2)
            qk3_fill = ([(lambda j=j: qk_tile(wq_sb, qT_sb, j, 3))
                         for j in range(HPG)] +
                        [(lambda j=j: qk_tile(wk_sb, kT_sb, j, 3))
                         for j in range(HPG)])
            attention(1, fillers=qk3_fill[:4])
            attention(2, fillers=qk3_fill[4:])
            vproj(3)
            # wo reuses wq's SBUF slot; triggers ride the Activation queue so
            # their wait (wq's last reader = qk(3)) can't block sync DMAs.
            wo_sb = load_w(woT, "wo", emit=False)
            for hh in range(KC):
                nc.scalar.dma_start(wo_sb[:, hh * OCG:(hh + 1) * OCG],
                                    woT[hh * 128:(hh + 1) * 128, :])
            at_tiles = [readback(0), readback(1)]
            attention(3)
            at_tiles += [readback(2), readback(3)]

            # ============ phase 3: o_proj (column-sharded, PE-only tail) ====
            # All inputs (at_tiles, readbacks issued during phase 2) are in
            # SBUF by now; this is pure PE work so the kernel tail is short.
            for tq in range(n_tq):
                at_sb = at_tiles[tq]
                for sub in range(TQ // 128):
                    pm = ps.tile([128, OCG], F32, tag="mm", bufs=2)
                    for i in range(n_group * HPG):
                        nc.tensor.matmul(
                            pm[:],
                            at_sb[i // 4][:, (i % 4) * TQ + sub * 128:
                                          (i % 4) * TQ + (sub + 1) * 128],
                            wo_sb[:, i * OCG:(i + 1) * OCG],
                            start=(i == 0), stop=(i == n_group * HPG - 1))
                    ob = pk.tile([128, OCG], BF16, tag="ob", bufs=6)
                    nc.vector.tensor_copy(ob[:], pm[:])
                    r0 = tq * TQ + sub * 128
                    # Activation HWDGE queue: the sync queue's tail is the
                    # at(3) readback triggers still waiting on AG(3).
                    nc.scalar.dma_start(out[r0:r0 + 128, :], ob[:])

    nc.compile()
    return nc


_NC_CACHE = {}


def _get_nc():
    if "nc" not in _NC_CACHE:
        _NC_CACHE["nc"] = build_nc()
    return _NC_CACHE["nc"]


def _build_in_maps(hidden_states, w_qkv, w_o):
    bf = ml_dtypes.bfloat16
    hidT = [np.ascontiguousarray(hidden_states[b].T).astype(bf) for b in range(B)]
    wq_all = w_qkv[:H].T.astype(bf)
    wk_all = w_qkv[H:2 * H].T.astype(bf)
    wv_all = w_qkv[2 * H:].T.astype(bf)
    wo_all = w_o.T.astype(bf)
    in_maps = []
    for c in range(NC):
        b, g = c // 4, c % 4
        sl = slice(g * OCG, (g + 1) * OCG)
        in_maps.append({
            "hidT": hidT[b],
            "wqT": np.ascontiguousarray(wq_all[:, sl]),
            "wkT": np.ascontiguousarray(wk_all[:, sl]),
            "wvT": np.ascontiguousarray(wv_all[:, sl]),
            "woT": np.ascontiguousarray(wo_all[:, sl]),
        })
    return in_maps


def kernel(hidden_states, w_qkv, w_o):
    hidden_states = np.asarray(hidden_states, dtype=np.float32)
    w_qkv = np.asarray(w_qkv, dtype=np.float32)
    w_o = np.asarray(w_o, dtype=np.float32)

    nc = _get_nc()
    in_maps = _build_in_maps(hidden_states, w_qkv, w_o)
    res = run_bass_kernel_spmd(nc, in_maps, core_ids=list(range(NC)))

    out = np.empty((B, S, H), np.float32)
    for c in range(NC):
        b, g = c // 4, c % 4
        out[b, :, g * OCG:(g + 1) * OCG] = \
            np.asarray(res.results[c]["out"], dtype=np.float32)
    return out

